# revision 6
# baseline (speedup 1.0000x reference)
"""Logsparse attention Trainium2 kernel.

Problem: B=4 H=8 L=4096 E=64, mask = causal & (dist <= win_len | dist is pow2).

Structure exploited: with 128-row query blocks b and 128-row key blocks,
query block b only interacts with key blocks {b, b-1, b-2, b-4, b-8, b-16}:
  - blocks b, b-1 carry the sliding window (win_len <= 127) plus pow2 dists
    {1..128} (dense-ish mask),
  - blocks b-2, b-4, b-8, b-16 carry exactly the pow2 dists 256/512/1024/2048,
    whose in-block mask is the pure diagonal kk == qq.

Sharding: B*H = 32 heads, 4 per core (8 cores). Heads processed in pairs:
the pair's [L, 2*64] q/k matrices are host-transposed into [128, L]
(e-on-partition) so QK^T matmuls contract over e; the two heads occupy
partition halves and run as row-packed (tile_position) concurrent matmuls.

Softmax: no max-subtraction (scores ~N(0,1), exp is safe in fp32/bf16);
denominator comes for free from a ones-column appended to V.
Compute dtype bf16 (inputs cast on host), accumulation fp32 (PSUM).

All HBM traffic is partition-major and contiguous per partition (v gets its
ones column and [128, 2, NB, 65] layout on the host; the output is stored
as [128, NB, 2, OUT_NB, 64] bf16 and un-permuted on the host) so DMAs don't
fragment into tiny descriptors. Input DMas are emitted need-first so the
first score matmul unblocks after ~3 small transfers.
"""

import os
import sys
from contextlib import ExitStack

import numpy as np

for _p in ("/opt/trn_rl_repo", "/root/.axon_site/_ro/trn_rl_repo"):
    if os.path.isdir(_p) and _p not in sys.path:
        sys.path.insert(0, _p)

import ml_dtypes  # noqa: E402
import concourse.bass as bass  # noqa: E402
import concourse.tile as tile  # noqa: E402
from concourse import bacc, mybir  # noqa: E402
from concourse.bass import ds  # noqa: E402
from concourse.bass_utils import run_bass_kernel_spmd  # noqa: E402

B, H, L, E = 4, 8, 4096, 64
NCORES = 8
BH = B * H                  # 32 heads total
BH_PER_CORE = BH // NCORES  # 4
NPAIRS = BH_PER_CORE // 2   # 2 head-pairs per core
NB = L // 128               # 32 query/key blocks
DELTAS = (0, 1, 2, 4, 8, 16)
NSLOT = len(DELTAS)
OUT_NB = 4                  # query blocks batched per output DMA
SCALE = 1.0 / float(np.sqrt(E))
BF16 = ml_dtypes.bfloat16

_NC_CACHE = {}


def _active(b):
    return [(j, d) for j, d in enumerate(DELTAS) if b - d >= 0]


def _kernel_body(ctx, tc, q2, k2, vext_in, maskt, out):
    nc = tc.nc
    consts = ctx.enter_context(tc.tile_pool(name="consts", bufs=1))
    pairbuf = ctx.enter_context(tc.tile_pool(name="pair", bufs=2))
    ppool = ctx.enter_context(tc.tile_pool(name="pexp", bufs=5))
    pmpool = ctx.enter_context(tc.tile_pool(name="pmask", bufs=5))
    spool = ctx.enter_context(tc.tile_pool(name="spsum", bufs=2, space="PSUM"))
    opool = ctx.enter_context(tc.tile_pool(name="opsum", bufs=2, space="PSUM"))
    rpool = ctx.enter_context(tc.tile_pool(name="rtile", bufs=4))
    outpool = ctx.enter_context(tc.tile_pool(name="outsb", bufs=3))

    mask_sb = consts.tile([128, 2, NSLOT, 128], mybir.dt.bfloat16)
    nc.sync.dma_start(out=mask_sb[:], in_=maskt[:])

    # Input loads in per-chunk TILES: dependency tracking is per-tile, so
    # item 0's matmuls only wait for the first small chunk rather than the
    # whole 4 MiB of qT/kT. Small head chunks first, need-ordered.
    CHUNKS = ((0, 1), (1, 2), (2, 4), (4, 8), (8, 16), (16, 24), (24, 32))
    qT_blk = [[None] * NB for _ in range(NPAIRS)]  # (tile, col_off) per block
    kT_blk = [[None] * NB for _ in range(NPAIRS)]
    v_blk = [[None] * NB for _ in range(NPAIRS)]  # (tile, blk_off) per block
    for lo, hi in CHUNKS:
        n = hi - lo
        for pr in range(NPAIRS):
            qt = pairbuf.tile([128, n * 128], mybir.dt.bfloat16, tag=f"qT{pr}_{lo}")
            kt = pairbuf.tile([128, n * 128], mybir.dt.bfloat16, tag=f"kT{pr}_{lo}")
            vt = pairbuf.tile([128, 2, n, 65], mybir.dt.bfloat16, tag=f"v{pr}_{lo}")
            nc.sync.dma_start(out=qt[:], in_=q2[pr][:, ds(lo * 128, n * 128)])
            nc.sync.dma_start(out=kt[:], in_=k2[pr][:, ds(lo * 128, n * 128)])
            nc.sync.dma_start(out=vt[:], in_=vext_in[pr][:, :, ds(lo, n), :])
            for b in range(lo, hi):
                qT_blk[pr][b] = (qt, (b - lo) * 128)
                kT_blk[pr][b] = (kt, (b - lo) * 128)
                v_blk[pr][b] = (vt, b - lo)

    # Software-pipelined emission (PE queue is in-order): scores of item t
    # are issued before exp/mask of t-1 and PV of t-2, so the PE always has
    # independent matmul work while ACT/DVE process earlier blocks.
    items = [(pr, b) for pr in range(NPAIRS) for b in range(NB)]
    st = {}
    out_sbs = {}

    def emit_scores(pr, b):
        S = spool.tile([128, 2, NSLOT, 128], mybir.dt.float32, tag="S")
        qt, qoff = qT_blk[pr][b]
        for j, d in _active(b):
            kt, koff = kT_blk[pr][b - d]
            for h in range(2):
                nc.tensor.matmul(
                    S[:, h, j, :],
                    lhsT=kt[64 * h : 64 * h + 64, ds(koff, 128)],
                    rhs=qt[64 * h : 64 * h + 64, ds(qoff, 128)],
                    start=True,
                    stop=True,
                    tile_position=(64 * h, 0),
                )
        st[(pr, b)] = S

    def emit_expmask(pr, b):
        S = st.pop((pr, b))
        P = ppool.tile([128, 2, NSLOT, 128], mybir.dt.bfloat16, tag="P")
        nc.scalar.activation(
            P[:], S[:], mybir.ActivationFunctionType.Exp, scale=SCALE
        )
        PM = pmpool.tile([128, 2, NSLOT, 128], mybir.dt.bfloat16, tag="PM")
        nc.vector.tensor_mul(PM[:], P[:], mask_sb[:])
        st[(pr, b, "PM")] = PM

    def emit_pv(pr, b):
        PM = st.pop((pr, b, "PM"))
        acts = _active(b)
        O = opool.tile([128, 2, 65], mybir.dt.float32, tag="O")
        for h in range(2):
            for i, (j, d) in enumerate(acts):
                vt, voff = v_blk[pr][b - d]
                nc.tensor.matmul(
                    O[:, h, :],
                    lhsT=PM[:, h, j, :],
                    rhs=vt[:, h, voff, :],
                    start=(i == 0),
                    stop=(i == len(acts) - 1),
                )
        r = rpool.tile([128, 2], mybir.dt.float32, tag="r")
        nc.vector.reciprocal(r[:], O[:, :, 64])
        if b % OUT_NB == 0:
            out_sbs[pr] = outpool.tile(
                [128, OUT_NB, 2, 64], mybir.dt.bfloat16, tag="osb", name="osb"
            )
        out_sb = out_sbs[pr]
        nc.vector.tensor_mul(
            out_sb[:, b % OUT_NB, :, :],
            O[:, :, 0:64],
            r[:].to_broadcast([128, 2, 64]),
        )
        if b % OUT_NB == OUT_NB - 1:
            w0 = b - (OUT_NB - 1)
            nc.sync.dma_start(
                out=out[pr][:, ds(w0, OUT_NB), :, :],
                in_=out_sb[:],
            )

    for t, (pr, b) in enumerate(items):
        emit_scores(pr, b)
        if t >= 1:
            emit_expmask(*items[t - 1])
        if t >= 2:
            emit_pv(*items[t - 2])
    emit_expmask(*items[-1])
    emit_pv(*items[-2])
    emit_pv(*items[-1])


def _build_nc():
    key = "v3"
    if key in _NC_CACHE:
        return _NC_CACHE[key]
    nc = bacc.Bacc(
        "TRN2",
        target_bir_lowering=False,
        debug=False,
        enable_asserts=False,
        num_devices=NCORES,
    )
    q2 = nc.dram_tensor("q2", [NPAIRS, 128, L], mybir.dt.bfloat16, kind="ExternalInput")
    k2 = nc.dram_tensor("k2", [NPAIRS, 128, L], mybir.dt.bfloat16, kind="ExternalInput")
    vext_in = nc.dram_tensor(
        "vext", [NPAIRS, 128, 2, NB, 65], mybir.dt.bfloat16, kind="ExternalInput"
    )
    maskt = nc.dram_tensor(
        "maskt", [128, 2, NSLOT, 128], mybir.dt.bfloat16, kind="ExternalInput"
    )
    out = nc.dram_tensor(
        "out", [NPAIRS, 128, NB, 2, 64], mybir.dt.bfloat16, kind="ExternalOutput"
    )
    with tile.TileContext(nc) as tc, ExitStack() as ctx:
        _kernel_body(ctx, tc, q2.ap(), k2.ap(), vext_in.ap(), maskt.ap(), out.ap())
    nc.compile()
    _NC_CACHE[key] = nc
    return nc


def _mask_tiles(win):
    kk = np.arange(128, dtype=np.int64)[:, None]
    qq = np.arange(128, dtype=np.int64)[None, :]
    tiles = np.zeros((128, 2, NSLOT, 128), np.float32)
    for j, d in enumerate(DELTAS):
        dist = 128 * d + qq - kk
        pow2 = (dist > 0) & ((dist & (dist - 1)) == 0)
        ok = (dist >= 0) & ((dist <= win) | pow2)
        tiles[:, 0, j, :] = ok
        tiles[:, 1, j, :] = ok
    return tiles.astype(BF16)


def _run(q, k, v, win_len, trace=False):
    win = int(np.asarray(win_len))
    assert 0 <= win < 128, f"win_len {win} out of supported range [0, 128)"
    q = np.asarray(q, dtype=np.float32).reshape(BH, L, E)
    k = np.asarray(k, dtype=np.float32).reshape(BH, L, E)
    v = np.asarray(v, dtype=np.float32).reshape(BH, L, E)
    maskt = _mask_tiles(win)

    in_maps = []
    for c in range(NCORES):
        sl = slice(BH_PER_CORE * c, BH_PER_CORE * (c + 1))
        qc = q[sl].astype(BF16)  # [4, L, E]
        kc = k[sl].astype(BF16)
        vc = v[sl].astype(BF16)
        # pack head pairs on partitions, pre-transposed: [pairs, (h e), L]
        q2 = np.ascontiguousarray(
            qc.reshape(NPAIRS, 2, L, E).transpose(0, 1, 3, 2).reshape(NPAIRS, 128, L)
        )
        k2 = np.ascontiguousarray(
            kc.reshape(NPAIRS, 2, L, E).transpose(0, 1, 3, 2).reshape(NPAIRS, 128, L)
        )
        # v packed partition-major with the ones column baked in:
        # [pr, 128, 2, NB, 65];  v row 128*n+p of head (pr,h) -> [pr, p, h, n, 0:64]
        vx = np.ones((NPAIRS, 2, NB, 128, 65), np.float32).astype(BF16)
        vx[:, :, :, :, 0:64] = vc.reshape(NPAIRS, 2, NB, 128, E)
        vext = np.ascontiguousarray(vx.transpose(0, 3, 1, 2, 4))
        in_maps.append({"q2": q2, "k2": k2, "vext": vext, "maskt": maskt})

    nc = _build_nc()
    res = run_bass_kernel_spmd(nc, in_maps, core_ids=list(range(NCORES)), trace=trace)
    # out_dev [pr, 128, NB, 2, 64] -> [pr, h, NB, 128, 64] -> [4, L, E]
    outs = np.stack(
        [
            np.asarray(res.results[c]["out"], dtype=np.float32)
            .transpose(0, 3, 2, 1, 4)
            .reshape(BH_PER_CORE, L, E)
            for c in range(NCORES)
        ]
    )
    full = outs.reshape(B, H, L, E)
    return full, res


def kernel(q, k, v, win_len):
    out, _ = _run(q, k, v, win_len, trace=False)
    return out


# revision 12
# speedup vs baseline: 1.0459x; 1.0459x over previous
"""Logsparse attention Trainium2 kernel.

Problem: B=4 H=8 L=4096 E=64, mask = causal & (dist <= win_len | dist is pow2).

Structure exploited: with 128-row query blocks b and 128-row key blocks,
query block b only interacts with key blocks {b, b-1, b-2, b-4, b-8, b-16}:
  - blocks b, b-1 carry the sliding window (win_len <= 127) plus pow2 dists
    {1..128} (dense-ish mask),
  - blocks b-2, b-4, b-8, b-16 carry exactly the pow2 dists 256/512/1024/2048,
    whose in-block mask is the pure diagonal kk == qq.

Sharding: B*H = 32 heads, 4 per core (8 cores). Heads processed in pairs:
the pair's [L, 2*64] q/k matrices are host-transposed into [128, L]
(e-on-partition) so QK^T matmuls contract over e; the two heads occupy
partition halves and run as row-packed (tile_position) concurrent matmuls.

Softmax: no max-subtraction (scores ~N(0,1), exp is safe in fp32/bf16);
denominator comes for free from a ones-column appended to V.
Compute dtype bf16 (inputs cast on host), accumulation fp32 (PSUM).

All HBM traffic is partition-major and contiguous per partition (v gets its
ones column and [128, 2, NB, 65] layout on the host; the output is stored
as [128, NB, 2, OUT_NB, 64] bf16 and un-permuted on the host) so DMAs don't
fragment into tiny descriptors. Input DMas are emitted need-first so the
first score matmul unblocks after ~3 small transfers.
"""

import os
import sys
from contextlib import ExitStack

import numpy as np

for _p in ("/opt/trn_rl_repo", "/root/.axon_site/_ro/trn_rl_repo"):
    if os.path.isdir(_p) and _p not in sys.path:
        sys.path.insert(0, _p)

import ml_dtypes  # noqa: E402
import concourse.bass as bass  # noqa: E402
import concourse.tile as tile  # noqa: E402
from concourse import bacc, mybir  # noqa: E402
from concourse.bass import ds  # noqa: E402
from concourse.bass_utils import run_bass_kernel_spmd  # noqa: E402

B, H, L, E = 4, 8, 4096, 64
NCORES = 8
BH = B * H                  # 32 heads total
BH_PER_CORE = BH // NCORES  # 4
NPAIRS = BH_PER_CORE // 2   # 2 head-pairs per core
NB = L // 128               # 32 query/key blocks
DELTAS = (0, 1, 2, 4, 8, 16)
NSLOT = len(DELTAS)
OUT_NB = 4                  # query blocks batched per output DMA
SCALE = 1.0 / float(np.sqrt(E))
BF16 = ml_dtypes.bfloat16

# input chunking (in 128-row blocks): small head chunks, need-ordered
CHUNKS = ((0, 1), (1, 2), (2, 4), (4, 8), (8, 20), (20, 32))
# columns per block in the packed per-chunk layout: q(128) + k(128) + v(2*65)
CPB = 128 + 128 + 2 * 65
PACK_COLS = NB * CPB

_NC_CACHE = {}


def _chunk_off(lo):
    return lo * CPB


def _chunk_lo(b):
    for lo, hi in CHUNKS:
        if lo <= b < hi:
            return lo
    raise ValueError(b)


def _active(b):
    return [(j, d) for j, d in enumerate(DELTAS) if b - d >= 0]


def _kernel_body(ctx, tc, inpk, maskt, out):
    nc = tc.nc
    consts = ctx.enter_context(tc.tile_pool(name="consts", bufs=1))
    pairbuf = ctx.enter_context(tc.tile_pool(name="pair", bufs=2))
    ppool = ctx.enter_context(tc.tile_pool(name="pexp", bufs=5))
    pmpool = ctx.enter_context(tc.tile_pool(name="pmask", bufs=5))
    spool = ctx.enter_context(tc.tile_pool(name="spsum", bufs=2, space="PSUM"))
    opool = ctx.enter_context(tc.tile_pool(name="opsum", bufs=2, space="PSUM"))
    rpool = ctx.enter_context(tc.tile_pool(name="rtile", bufs=4))
    outpool = ctx.enter_context(tc.tile_pool(name="outsb", bufs=3))

    # Input loads: q|k|v packed per chunk into ONE host-packed region so each
    # chunk is a single DMA trigger (HWDGE triggers cost ~0.6us each on the
    # sync queue and serialize). Per-chunk TILES keep dependency tracking
    # fine-grained: item 0's matmuls only wait for the first small chunk.
    # pr0's chunks are all emitted before pr1's (items are pr-major).
    qT_blk = [[None] * NB for _ in range(NPAIRS)]  # (tile, col_off) per block
    kT_blk = [[None] * NB for _ in range(NPAIRS)]
    v_blk = [[None] * NB for _ in range(NPAIRS)]  # (tile, col_off) per block
    mask_sb = consts.tile([128, 2, NSLOT, 128], mybir.dt.bfloat16)
    first = True
    for pr in range(NPAIRS):
        for lo, hi in CHUNKS:
            n = hi - lo
            sz = n * CPB
            t = pairbuf.tile([128, sz], mybir.dt.bfloat16, tag=f"in{pr}_{lo}")
            off = _chunk_off(lo)
            nc.sync.dma_start(out=t[:], in_=inpk[pr][:, ds(off, sz)])
            if first:
                # mask is first needed by item 0's expmask, after chunk 0
                nc.sync.dma_start(out=mask_sb[:], in_=maskt[:])
                first = False
            for b in range(lo, hi):
                i = b - lo
                qT_blk[pr][b] = (t, i * 128)
                kT_blk[pr][b] = (t, n * 128 + i * 128)
                # v block for head h at: 2*n*128 + h*(n*65) + i*65
                v_blk[pr][b] = (t, 2 * n * 128, n)

    # Software-pipelined emission (PE queue is in-order): scores of item t
    # are issued before exp/mask of t-1 and PV of t-2, so the PE always has
    # independent matmul work while ACT/DVE process earlier blocks.
    items = [(pr, b) for pr in range(NPAIRS) for b in range(NB)]
    st = {}
    out_sbs = {}

    def emit_scores(pr, b):
        S = spool.tile([128, 2, NSLOT, 128], mybir.dt.float32, tag="S")
        qt, qoff = qT_blk[pr][b]
        for j, d in _active(b):
            kt, koff = kT_blk[pr][b - d]
            for h in range(2):
                nc.tensor.matmul(
                    S[:, h, j, :],
                    lhsT=kt[64 * h : 64 * h + 64, ds(koff, 128)],
                    rhs=qt[64 * h : 64 * h + 64, ds(qoff, 128)],
                    start=True,
                    stop=True,
                    tile_position=(64 * h, 0),
                )
        st[(pr, b)] = S

    def emit_expmask(pr, b):
        S = st.pop((pr, b))
        P = ppool.tile([128, 2, NSLOT, 128], mybir.dt.bfloat16, tag="P")
        nc.scalar.activation(
            P[:], S[:], mybir.ActivationFunctionType.Exp, scale=SCALE
        )
        PM = pmpool.tile([128, 2, NSLOT, 128], mybir.dt.bfloat16, tag="PM")
        nc.vector.tensor_mul(PM[:], P[:], mask_sb[:])
        st[(pr, b, "PM")] = PM

    def emit_pv(pr, b):
        PM = st.pop((pr, b, "PM"))
        acts = _active(b)
        O = opool.tile([128, 2, 65], mybir.dt.float32, tag="O")
        for h in range(2):
            for i, (j, d) in enumerate(acts):
                vt, vbase, n = v_blk[pr][b - d]
                voff = vbase + h * (n * 65) + (b - d - _chunk_lo(b - d)) * 65
                nc.tensor.matmul(
                    O[:, h, :],
                    lhsT=PM[:, h, j, :],
                    rhs=vt[:, ds(voff, 65)],
                    start=(i == 0),
                    stop=(i == len(acts) - 1),
                )
        r = rpool.tile([128, 2], mybir.dt.float32, tag="r")
        nc.vector.reciprocal(r[:], O[:, :, 64])
        if b % OUT_NB == 0:
            out_sbs[pr] = outpool.tile(
                [128, OUT_NB, 2, 64], mybir.dt.bfloat16, tag="osb", name="osb"
            )
        out_sb = out_sbs[pr]
        nc.vector.tensor_mul(
            out_sb[:, b % OUT_NB, :, :],
            O[:, :, 0:64],
            r[:].to_broadcast([128, 2, 64]),
        )
        if b % OUT_NB == OUT_NB - 1:
            w0 = b - (OUT_NB - 1)
            nc.sync.dma_start(
                out=out[pr][:, ds(w0, OUT_NB), :, :],
                in_=out_sb[:],
            )

    for t, (pr, b) in enumerate(items):
        emit_scores(pr, b)
        if t >= 1:
            emit_expmask(*items[t - 1])
        if t >= 2:
            emit_pv(*items[t - 2])
    emit_expmask(*items[-1])
    emit_pv(*items[-2])
    emit_pv(*items[-1])


def _build_nc():
    key = "v4"
    if key in _NC_CACHE:
        return _NC_CACHE[key]
    nc = bacc.Bacc(
        "TRN2",
        target_bir_lowering=False,
        debug=False,
        enable_asserts=False,
        num_devices=NCORES,
    )
    inpk = nc.dram_tensor(
        "inpk", [NPAIRS, 128, PACK_COLS], mybir.dt.bfloat16, kind="ExternalInput"
    )
    maskt = nc.dram_tensor(
        "maskt", [128, 2, NSLOT, 128], mybir.dt.bfloat16, kind="ExternalInput"
    )
    out = nc.dram_tensor(
        "out", [NPAIRS, 128, NB, 2, 64], mybir.dt.bfloat16, kind="ExternalOutput"
    )
    with tile.TileContext(nc) as tc, ExitStack() as ctx:
        _kernel_body(ctx, tc, inpk.ap(), maskt.ap(), out.ap())
    nc.compile()
    _NC_CACHE[key] = nc
    return nc


def _mask_tiles(win):
    kk = np.arange(128, dtype=np.int64)[:, None]
    qq = np.arange(128, dtype=np.int64)[None, :]
    tiles = np.zeros((128, 2, NSLOT, 128), np.float32)
    for j, d in enumerate(DELTAS):
        dist = 128 * d + qq - kk
        pow2 = (dist > 0) & ((dist & (dist - 1)) == 0)
        ok = (dist >= 0) & ((dist <= win) | pow2)
        tiles[:, 0, j, :] = ok
        tiles[:, 1, j, :] = ok
    return tiles.astype(BF16)


def _run(q, k, v, win_len, trace=False):
    win = int(np.asarray(win_len))
    assert 0 <= win < 128, f"win_len {win} out of supported range [0, 128)"
    q = np.asarray(q, dtype=np.float32).reshape(BH, L, E)
    k = np.asarray(k, dtype=np.float32).reshape(BH, L, E)
    v = np.asarray(v, dtype=np.float32).reshape(BH, L, E)
    maskt = _mask_tiles(win)

    in_maps = []
    for c in range(NCORES):
        sl = slice(BH_PER_CORE * c, BH_PER_CORE * (c + 1))
        qc = q[sl].astype(BF16)  # [4, L, E]
        kc = k[sl].astype(BF16)
        vc = v[sl].astype(BF16)
        # pack head pairs on partitions, pre-transposed: [pairs, (h e), L]
        q2 = qc.reshape(NPAIRS, 2, L, E).transpose(0, 1, 3, 2).reshape(NPAIRS, 128, L)
        k2 = kc.reshape(NPAIRS, 2, L, E).transpose(0, 1, 3, 2).reshape(NPAIRS, 128, L)
        # v partition-major with the ones column baked in:
        # [pr, 128, 2, NB, 65];  v row 128*n+p of head (pr,h) -> [pr, p, h, n, 0:64]
        vx = np.ones((NPAIRS, 2, NB, 128, 65), np.float32).astype(BF16)
        vx[:, :, :, :, 0:64] = vc.reshape(NPAIRS, 2, NB, 128, E)
        vext = vx.transpose(0, 3, 1, 2, 4)  # [pr, 128, 2, NB, 65]
        # packed per-chunk input: for each chunk [q cols | k cols | v blocks]
        inpk = np.empty((NPAIRS, 128, PACK_COLS), BF16)
        for lo, hi in CHUNKS:
            n = hi - lo
            off = _chunk_off(lo)
            inpk[:, :, off : off + n * 128] = q2[:, :, lo * 128 : hi * 128]
            off += n * 128
            inpk[:, :, off : off + n * 128] = k2[:, :, lo * 128 : hi * 128]
            off += n * 128
            inpk[:, :, off : off + 2 * n * 65] = vext[:, :, :, lo:hi, :].reshape(
                NPAIRS, 128, 2 * n * 65
            )
        in_maps.append({"inpk": np.ascontiguousarray(inpk), "maskt": maskt})

    nc = _build_nc()
    res = run_bass_kernel_spmd(nc, in_maps, core_ids=list(range(NCORES)), trace=trace)
    # out_dev [pr, 128, NB, 2, 64] -> [pr, h, NB, 128, 64] -> [4, L, E]
    outs = np.stack(
        [
            np.asarray(res.results[c]["out"], dtype=np.float32)
            .transpose(0, 3, 2, 1, 4)
            .reshape(BH_PER_CORE, L, E)
            for c in range(NCORES)
        ]
    )
    full = outs.reshape(B, H, L, E)
    return full, res


def kernel(q, k, v, win_len):
    out, _ = _run(q, k, v, win_len, trace=False)
    return out


# revision 13
# speedup vs baseline: 1.0759x; 1.0287x over previous
"""Logsparse attention Trainium2 kernel.

Problem: B=4 H=8 L=4096 E=64, mask = causal & (dist <= win_len | dist is pow2).

Structure exploited: with 128-row query blocks b and 128-row key blocks,
query block b only interacts with key blocks {b, b-1, b-2, b-4, b-8, b-16}:
  - blocks b, b-1 carry the sliding window (win_len <= 127) plus pow2 dists
    {1..128} (dense-ish mask),
  - blocks b-2, b-4, b-8, b-16 carry exactly the pow2 dists 256/512/1024/2048,
    whose in-block mask is the pure diagonal kk == qq.

Sharding: B*H = 32 heads, 4 per core (8 cores). Heads processed in pairs:
the pair's [L, 2*64] q/k matrices are host-transposed into [128, L]
(e-on-partition) so QK^T matmuls contract over e; the two heads occupy
partition halves and run as row-packed (tile_position) concurrent matmuls.

Softmax: no max-subtraction (scores ~N(0,1), exp is safe in fp32/bf16);
denominator comes for free from a ones-column appended to V.
Compute dtype bf16 (inputs cast on host), accumulation fp32 (PSUM).

All HBM traffic is partition-major and contiguous per partition (v gets its
ones column and [128, 2, NB, 65] layout on the host; the output is stored
as [128, NB, 2, OUT_NB, 64] bf16 and un-permuted on the host) so DMAs don't
fragment into tiny descriptors. Input DMas are emitted need-first so the
first score matmul unblocks after ~3 small transfers.
"""

import os
import sys
from contextlib import ExitStack

import numpy as np

for _p in ("/opt/trn_rl_repo", "/root/.axon_site/_ro/trn_rl_repo"):
    if os.path.isdir(_p) and _p not in sys.path:
        sys.path.insert(0, _p)

import ml_dtypes  # noqa: E402
import concourse.bass as bass  # noqa: E402
import concourse.tile as tile  # noqa: E402
from concourse import bacc, mybir  # noqa: E402
from concourse.bass import ds  # noqa: E402
from concourse.bass_utils import run_bass_kernel_spmd  # noqa: E402

B, H, L, E = 4, 8, 4096, 64
NCORES = 8
BH = B * H                  # 32 heads total
BH_PER_CORE = BH // NCORES  # 4
NPAIRS = BH_PER_CORE // 2   # 2 head-pairs per core
NB = L // 128               # 32 query/key blocks
DELTAS = (0, 1, 2, 4, 8, 16)
NSLOT = len(DELTAS)
OUT_NB = 2                  # query blocks batched per output DMA
SCALE = 1.0 / float(np.sqrt(E))
BF16 = ml_dtypes.bfloat16

# input chunking (in 128-row blocks): small head chunks, need-ordered
CHUNKS = ((0, 2), (2, 4), (4, 8), (8, 20), (20, 32))
# columns per block in the packed per-chunk layout: q(128) + k(128) + v(2*65)
CPB = 128 + 128 + 2 * 65
PACK_COLS = NB * CPB

_NC_CACHE = {}


def _chunk_off(lo):
    return lo * CPB


def _chunk_lo(b):
    for lo, hi in CHUNKS:
        if lo <= b < hi:
            return lo
    raise ValueError(b)


def _active(b):
    return [(j, d) for j, d in enumerate(DELTAS) if b - d >= 0]


def _kernel_body(ctx, tc, inpk, maskt, out):
    nc = tc.nc
    consts = ctx.enter_context(tc.tile_pool(name="consts", bufs=1))
    pairbuf = ctx.enter_context(tc.tile_pool(name="pair", bufs=2))
    ppool = ctx.enter_context(tc.tile_pool(name="pexp", bufs=5))
    pmpool = ctx.enter_context(tc.tile_pool(name="pmask", bufs=5))
    spool = ctx.enter_context(tc.tile_pool(name="spsum", bufs=2, space="PSUM"))
    opool = ctx.enter_context(tc.tile_pool(name="opsum", bufs=2, space="PSUM"))
    rpool = ctx.enter_context(tc.tile_pool(name="rtile", bufs=4))
    outpool = ctx.enter_context(tc.tile_pool(name="outsb", bufs=3))

    # Input loads: q|k|v packed per chunk into ONE host-packed region so each
    # chunk is a single DMA trigger (HWDGE triggers cost ~0.6us each on the
    # sync queue and serialize). Per-chunk TILES keep dependency tracking
    # fine-grained: item 0's matmuls only wait for the first small chunk.
    # pr0's chunks are all emitted before pr1's (items are pr-major).
    qT_blk = [[None] * NB for _ in range(NPAIRS)]  # (tile, col_off) per block
    kT_blk = [[None] * NB for _ in range(NPAIRS)]
    v_blk = [[None] * NB for _ in range(NPAIRS)]  # (tile, col_off) per block
    mask_sb = consts.tile([128, 2, NSLOT, 128], mybir.dt.bfloat16)
    first = True
    for pr in range(NPAIRS):
        for lo, hi in CHUNKS:
            n = hi - lo
            sz = n * CPB
            t = pairbuf.tile([128, sz], mybir.dt.bfloat16, tag=f"in{pr}_{lo}")
            off = _chunk_off(lo)
            nc.sync.dma_start(out=t[:], in_=inpk[pr][:, ds(off, sz)])
            if first:
                # mask is first needed by item 0's expmask, after chunk 0
                nc.sync.dma_start(out=mask_sb[:], in_=maskt[:])
                first = False
            for b in range(lo, hi):
                i = b - lo
                qT_blk[pr][b] = (t, i * 128)
                kT_blk[pr][b] = (t, n * 128 + i * 128)
                # v block for head h at: 2*n*128 + h*(n*65) + i*65
                v_blk[pr][b] = (t, 2 * n * 128, n)

    # Software-pipelined emission (PE queue is in-order): scores of item t
    # are issued before exp/mask of t-1 and PV of t-2, so the PE always has
    # independent matmul work while ACT/DVE process earlier blocks.
    items = [(pr, b) for pr in range(NPAIRS) for b in range(NB)]
    st = {}
    out_sbs = {}

    def emit_scores(pr, b):
        S = spool.tile([128, 2, NSLOT, 128], mybir.dt.float32, tag="S")
        qt, qoff = qT_blk[pr][b]
        for j, d in _active(b):
            kt, koff = kT_blk[pr][b - d]
            for h in range(2):
                nc.tensor.matmul(
                    S[:, h, j, :],
                    lhsT=kt[64 * h : 64 * h + 64, ds(koff, 128)],
                    rhs=qt[64 * h : 64 * h + 64, ds(qoff, 128)],
                    start=True,
                    stop=True,
                    tile_position=(64 * h, 0),
                )
        st[(pr, b)] = S

    def emit_expmask(pr, b):
        S = st.pop((pr, b))
        P = ppool.tile([128, 2, NSLOT, 128], mybir.dt.bfloat16, tag="P")
        nc.scalar.activation(
            P[:], S[:], mybir.ActivationFunctionType.Exp, scale=SCALE
        )
        PM = pmpool.tile([128, 2, NSLOT, 128], mybir.dt.bfloat16, tag="PM")
        nc.vector.tensor_mul(PM[:], P[:], mask_sb[:])
        st[(pr, b, "PM")] = PM

    def emit_pv(pr, b):
        PM = st.pop((pr, b, "PM"))
        acts = _active(b)
        O = opool.tile([128, 2, 65], mybir.dt.float32, tag="O")
        for h in range(2):
            for i, (j, d) in enumerate(acts):
                vt, vbase, n = v_blk[pr][b - d]
                voff = vbase + h * (n * 65) + (b - d - _chunk_lo(b - d)) * 65
                nc.tensor.matmul(
                    O[:, h, :],
                    lhsT=PM[:, h, j, :],
                    rhs=vt[:, ds(voff, 65)],
                    start=(i == 0),
                    stop=(i == len(acts) - 1),
                )
        r = rpool.tile([128, 2], mybir.dt.float32, tag="r")
        nc.vector.reciprocal(r[:], O[:, :, 64])
        if b % OUT_NB == 0:
            out_sbs[pr] = outpool.tile(
                [128, OUT_NB, 2, 64], mybir.dt.bfloat16, tag="osb", name="osb"
            )
        out_sb = out_sbs[pr]
        nc.vector.tensor_mul(
            out_sb[:, b % OUT_NB, :, :],
            O[:, :, 0:64],
            r[:].to_broadcast([128, 2, 64]),
        )
        if b % OUT_NB == OUT_NB - 1:
            w0 = b - (OUT_NB - 1)
            nc.sync.dma_start(
                out=out[pr][:, ds(w0, OUT_NB), :, :],
                in_=out_sb[:],
            )

    for t, (pr, b) in enumerate(items):
        emit_scores(pr, b)
        if t >= 1:
            emit_expmask(*items[t - 1])
        if t >= 2:
            emit_pv(*items[t - 2])
    emit_expmask(*items[-1])
    emit_pv(*items[-2])
    emit_pv(*items[-1])


def _build_nc():
    key = "v5"
    if key in _NC_CACHE:
        return _NC_CACHE[key]
    nc = bacc.Bacc(
        "TRN2",
        target_bir_lowering=False,
        debug=False,
        enable_asserts=False,
        num_devices=NCORES,
    )
    inpk = nc.dram_tensor(
        "inpk", [NPAIRS, 128, PACK_COLS], mybir.dt.bfloat16, kind="ExternalInput"
    )
    maskt = nc.dram_tensor(
        "maskt", [128, 2, NSLOT, 128], mybir.dt.bfloat16, kind="ExternalInput"
    )
    out = nc.dram_tensor(
        "out", [NPAIRS, 128, NB, 2, 64], mybir.dt.bfloat16, kind="ExternalOutput"
    )
    with tile.TileContext(nc) as tc, ExitStack() as ctx:
        _kernel_body(ctx, tc, inpk.ap(), maskt.ap(), out.ap())
    nc.compile()
    _NC_CACHE[key] = nc
    return nc


def _mask_tiles(win):
    kk = np.arange(128, dtype=np.int64)[:, None]
    qq = np.arange(128, dtype=np.int64)[None, :]
    tiles = np.zeros((128, 2, NSLOT, 128), np.float32)
    for j, d in enumerate(DELTAS):
        dist = 128 * d + qq - kk
        pow2 = (dist > 0) & ((dist & (dist - 1)) == 0)
        ok = (dist >= 0) & ((dist <= win) | pow2)
        tiles[:, 0, j, :] = ok
        tiles[:, 1, j, :] = ok
    return tiles.astype(BF16)


def _run(q, k, v, win_len, trace=False):
    win = int(np.asarray(win_len))
    assert 0 <= win < 128, f"win_len {win} out of supported range [0, 128)"
    q = np.asarray(q, dtype=np.float32).reshape(BH, L, E)
    k = np.asarray(k, dtype=np.float32).reshape(BH, L, E)
    v = np.asarray(v, dtype=np.float32).reshape(BH, L, E)
    maskt = _mask_tiles(win)

    in_maps = []
    for c in range(NCORES):
        sl = slice(BH_PER_CORE * c, BH_PER_CORE * (c + 1))
        qc = q[sl].astype(BF16)  # [4, L, E]
        kc = k[sl].astype(BF16)
        vc = v[sl].astype(BF16)
        # pack head pairs on partitions, pre-transposed: [pairs, (h e), L]
        q2 = qc.reshape(NPAIRS, 2, L, E).transpose(0, 1, 3, 2).reshape(NPAIRS, 128, L)
        k2 = kc.reshape(NPAIRS, 2, L, E).transpose(0, 1, 3, 2).reshape(NPAIRS, 128, L)
        # v partition-major with the ones column baked in:
        # [pr, 128, 2, NB, 65];  v row 128*n+p of head (pr,h) -> [pr, p, h, n, 0:64]
        vx = np.ones((NPAIRS, 2, NB, 128, 65), np.float32).astype(BF16)
        vx[:, :, :, :, 0:64] = vc.reshape(NPAIRS, 2, NB, 128, E)
        vext = vx.transpose(0, 3, 1, 2, 4)  # [pr, 128, 2, NB, 65]
        # packed per-chunk input: for each chunk [q cols | k cols | v blocks]
        inpk = np.empty((NPAIRS, 128, PACK_COLS), BF16)
        for lo, hi in CHUNKS:
            n = hi - lo
            off = _chunk_off(lo)
            inpk[:, :, off : off + n * 128] = q2[:, :, lo * 128 : hi * 128]
            off += n * 128
            inpk[:, :, off : off + n * 128] = k2[:, :, lo * 128 : hi * 128]
            off += n * 128
            inpk[:, :, off : off + 2 * n * 65] = vext[:, :, :, lo:hi, :].reshape(
                NPAIRS, 128, 2 * n * 65
            )
        in_maps.append({"inpk": np.ascontiguousarray(inpk), "maskt": maskt})

    nc = _build_nc()
    res = run_bass_kernel_spmd(nc, in_maps, core_ids=list(range(NCORES)), trace=trace)
    # out_dev [pr, 128, NB, 2, 64] -> [pr, h, NB, 128, 64] -> [4, L, E]
    outs = np.stack(
        [
            np.asarray(res.results[c]["out"], dtype=np.float32)
            .transpose(0, 3, 2, 1, 4)
            .reshape(BH_PER_CORE, L, E)
            for c in range(NCORES)
        ]
    )
    full = outs.reshape(B, H, L, E)
    return full, res


def kernel(q, k, v, win_len):
    out, _ = _run(q, k, v, win_len, trace=False)
    return out


# revision 15
# speedup vs baseline: 1.1048x; 1.0268x over previous
"""Logsparse attention Trainium2 kernel.

Problem: B=4 H=8 L=4096 E=64, mask = causal & (dist <= win_len | dist is pow2).

Structure exploited: with 128-row query blocks b and 128-row key blocks,
query block b only interacts with key blocks {b, b-1, b-2, b-4, b-8, b-16}:
  - blocks b, b-1 carry the sliding window (win_len <= 127) plus pow2 dists
    {1..128} (dense-ish mask),
  - blocks b-2, b-4, b-8, b-16 carry exactly the pow2 dists 256/512/1024/2048,
    whose in-block mask is the pure diagonal kk == qq.

Sharding: B*H = 32 heads, 4 per core (8 cores). Heads processed in pairs:
the pair's [L, 2*64] q/k matrices are host-transposed into [128, L]
(e-on-partition) so QK^T matmuls contract over e; the two heads occupy
partition halves and run as row-packed (tile_position) concurrent matmuls.

Softmax: no max-subtraction (scores ~N(0,1), exp is safe in fp32/bf16);
denominator comes for free from a ones-column appended to V.
Compute dtype bf16 (inputs cast on host), accumulation fp32 (PSUM).

All HBM traffic is partition-major and contiguous per partition (v gets its
ones column and [128, 2, NB, 65] layout on the host; the output is stored
as [128, NB, 2, OUT_NB, 64] bf16 and un-permuted on the host) so DMAs don't
fragment into tiny descriptors. Input DMas are emitted need-first so the
first score matmul unblocks after ~3 small transfers.
"""

import os
import sys
from contextlib import ExitStack

import numpy as np

for _p in ("/opt/trn_rl_repo", "/root/.axon_site/_ro/trn_rl_repo"):
    if os.path.isdir(_p) and _p not in sys.path:
        sys.path.insert(0, _p)

import ml_dtypes  # noqa: E402
import concourse.bass as bass  # noqa: E402
import concourse.tile as tile  # noqa: E402
from concourse import bacc, mybir  # noqa: E402
from concourse.bass import ds  # noqa: E402
from concourse.bass_utils import run_bass_kernel_spmd  # noqa: E402

B, H, L, E = 4, 8, 4096, 64
NCORES = 8
BH = B * H                  # 32 heads total
BH_PER_CORE = BH // NCORES  # 4
NPAIRS = BH_PER_CORE // 2   # 2 head-pairs per core
NB = L // 128               # 32 query/key blocks
DELTAS = (0, 1, 2, 4, 8, 16)
NSLOT = len(DELTAS)
OUT_NB = 2                  # query blocks batched per output DMA
SCALE = 1.0 / float(np.sqrt(E))
BF16 = ml_dtypes.bfloat16

# input chunking (in 128-row blocks): small head chunks, need-ordered
CHUNKS = ((0, 2), (2, 4), (4, 8), (8, 20), (20, 32))
# columns per block in the packed per-chunk layout: q(128) + k(128) + v(2*65)
CPB = 128 + 128 + 2 * 65
PACK_COLS = NB * CPB

_NC_CACHE = {}


def _chunk_off(lo):
    return lo * CPB


def _chunk_lo(b):
    for lo, hi in CHUNKS:
        if lo <= b < hi:
            return lo
    raise ValueError(b)


def _active(b):
    return [(j, d) for j, d in enumerate(DELTAS) if b - d >= 0]


def _kernel_body(ctx, tc, inpk, maskt, out):
    nc = tc.nc
    consts = ctx.enter_context(tc.tile_pool(name="consts", bufs=1))
    pairbuf = ctx.enter_context(tc.tile_pool(name="pair", bufs=2))
    ppool = ctx.enter_context(tc.tile_pool(name="pexp", bufs=5))
    pmpool = ctx.enter_context(tc.tile_pool(name="pmask", bufs=5))
    spool = ctx.enter_context(tc.tile_pool(name="spsum", bufs=2, space="PSUM"))
    opool = ctx.enter_context(tc.tile_pool(name="opsum", bufs=2, space="PSUM"))
    rpool = ctx.enter_context(tc.tile_pool(name="rtile", bufs=4))
    outpool = ctx.enter_context(tc.tile_pool(name="outsb", bufs=3))

    # Input loads: q|k|v packed per chunk into ONE host-packed region so each
    # chunk is a single DMA trigger (HWDGE triggers cost ~0.6us each on the
    # sync queue and serialize). Per-chunk TILES keep dependency tracking
    # fine-grained: item 0's matmuls only wait for the first small chunk.
    # pr0's chunks are all emitted before pr1's (items are pr-major).
    qT_blk = [[None] * NB for _ in range(NPAIRS)]  # (tile, col_off) per block
    kT_blk = [[None] * NB for _ in range(NPAIRS)]
    v_blk = [[None] * NB for _ in range(NPAIRS)]  # (tile, col_off) per block
    mask_sb = consts.tile([128, 2, NSLOT, 128], mybir.dt.bfloat16)
    first = True
    for pr in range(NPAIRS):
        for lo, hi in CHUNKS:
            n = hi - lo
            sz = n * CPB
            t = pairbuf.tile([128, sz], mybir.dt.bfloat16, tag=f"in{pr}_{lo}")
            off = _chunk_off(lo)
            nc.sync.dma_start(out=t[:], in_=inpk[pr][:, ds(off, sz)])
            if first:
                # mask is first needed by item 0's expmask, after chunk 0
                nc.sync.dma_start(out=mask_sb[:], in_=maskt[:])
                first = False
            for b in range(lo, hi):
                i = b - lo
                qT_blk[pr][b] = (t, i * 128)
                kT_blk[pr][b] = (t, n * 128 + i * 128)
                # v block for head h at: 2*n*128 + h*(n*65) + i*65
                v_blk[pr][b] = (t, 2 * n * 128, n)

    # Software-pipelined emission (PE queue is in-order): scores of item t
    # are issued before exp/mask of t-1 and PV of t-2, so the PE always has
    # independent matmul work while ACT/DVE process earlier blocks.
    items = [(pr, b) for pr in range(NPAIRS) for b in range(NB)]
    st = {}
    out_sbs = {}

    def emit_scores(pr, b):
        S = spool.tile([128, 2, NSLOT, 128], mybir.dt.float32, tag="S")
        qt, qoff = qT_blk[pr][b]
        for j, d in _active(b):
            kt, koff = kT_blk[pr][b - d]
            for h in range(2):
                nc.tensor.matmul(
                    S[:, h, j, :],
                    lhsT=kt[64 * h : 64 * h + 64, ds(koff, 128)],
                    rhs=qt[64 * h : 64 * h + 64, ds(qoff, 128)],
                    start=True,
                    stop=True,
                    tile_position=(64 * h, 0),
                )
        st[(pr, b)] = S

    def emit_expmask(pr, b):
        S = st.pop((pr, b))
        # active slots form a prefix (DELTAS ascending): only exp/mask those
        na = len(_active(b))
        P = ppool.tile([128, 2, NSLOT, 128], mybir.dt.bfloat16, tag="P")
        nc.scalar.activation(
            P[:, :, 0:na, :], S[:, :, 0:na, :],
            mybir.ActivationFunctionType.Exp, scale=SCALE,
        )
        PM = pmpool.tile([128, 2, NSLOT, 128], mybir.dt.bfloat16, tag="PM")
        nc.vector.tensor_mul(PM[:, :, 0:na, :], P[:, :, 0:na, :], mask_sb[:, :, 0:na, :])
        st[(pr, b, "PM")] = PM

    def emit_pv(pr, b):
        PM = st.pop((pr, b, "PM"))
        acts = _active(b)
        O = opool.tile([128, 2, 65], mybir.dt.float32, tag="O")
        for h in range(2):
            for i, (j, d) in enumerate(acts):
                vt, vbase, n = v_blk[pr][b - d]
                voff = vbase + h * (n * 65) + (b - d - _chunk_lo(b - d)) * 65
                nc.tensor.matmul(
                    O[:, h, :],
                    lhsT=PM[:, h, j, :],
                    rhs=vt[:, ds(voff, 65)],
                    start=(i == 0),
                    stop=(i == len(acts) - 1),
                )
        r = rpool.tile([128, 2], mybir.dt.float32, tag="r")
        nc.vector.reciprocal(r[:], O[:, :, 64])
        if b % OUT_NB == 0:
            out_sbs[pr] = outpool.tile(
                [128, OUT_NB, 2, 64], mybir.dt.bfloat16, tag="osb", name="osb"
            )
        out_sb = out_sbs[pr]
        nc.vector.tensor_mul(
            out_sb[:, b % OUT_NB, :, :],
            O[:, :, 0:64],
            r[:].to_broadcast([128, 2, 64]),
        )
        if b % OUT_NB == OUT_NB - 1:
            w0 = b - (OUT_NB - 1)
            nc.sync.dma_start(
                out=out[pr][:, ds(w0, OUT_NB), :, :],
                in_=out_sb[:],
            )

    for t, (pr, b) in enumerate(items):
        emit_scores(pr, b)
        if t >= 1:
            emit_expmask(*items[t - 1])
        if t >= 2:
            emit_pv(*items[t - 2])
    emit_expmask(*items[-1])
    emit_pv(*items[-2])
    emit_pv(*items[-1])


def _build_nc():
    key = "v6"
    if key in _NC_CACHE:
        return _NC_CACHE[key]
    nc = bacc.Bacc(
        "TRN2",
        target_bir_lowering=False,
        debug=False,
        enable_asserts=False,
        num_devices=NCORES,
    )
    inpk = nc.dram_tensor(
        "inpk", [NPAIRS, 128, PACK_COLS], mybir.dt.bfloat16, kind="ExternalInput"
    )
    maskt = nc.dram_tensor(
        "maskt", [128, 2, NSLOT, 128], mybir.dt.bfloat16, kind="ExternalInput"
    )
    out = nc.dram_tensor(
        "out", [NPAIRS, 128, NB, 2, 64], mybir.dt.bfloat16, kind="ExternalOutput"
    )
    with tile.TileContext(nc) as tc, ExitStack() as ctx:
        _kernel_body(ctx, tc, inpk.ap(), maskt.ap(), out.ap())
    nc.compile()
    _NC_CACHE[key] = nc
    return nc


def _mask_tiles(win):
    kk = np.arange(128, dtype=np.int64)[:, None]
    qq = np.arange(128, dtype=np.int64)[None, :]
    tiles = np.zeros((128, 2, NSLOT, 128), np.float32)
    for j, d in enumerate(DELTAS):
        dist = 128 * d + qq - kk
        pow2 = (dist > 0) & ((dist & (dist - 1)) == 0)
        ok = (dist >= 0) & ((dist <= win) | pow2)
        tiles[:, 0, j, :] = ok
        tiles[:, 1, j, :] = ok
    return tiles.astype(BF16)


def _run(q, k, v, win_len, trace=False):
    win = int(np.asarray(win_len))
    assert 0 <= win < 128, f"win_len {win} out of supported range [0, 128)"
    q = np.asarray(q, dtype=np.float32).reshape(BH, L, E)
    k = np.asarray(k, dtype=np.float32).reshape(BH, L, E)
    v = np.asarray(v, dtype=np.float32).reshape(BH, L, E)
    maskt = _mask_tiles(win)

    in_maps = []
    for c in range(NCORES):
        sl = slice(BH_PER_CORE * c, BH_PER_CORE * (c + 1))
        qc = q[sl].astype(BF16)  # [4, L, E]
        kc = k[sl].astype(BF16)
        vc = v[sl].astype(BF16)
        # pack head pairs on partitions, pre-transposed: [pairs, (h e), L]
        q2 = qc.reshape(NPAIRS, 2, L, E).transpose(0, 1, 3, 2).reshape(NPAIRS, 128, L)
        k2 = kc.reshape(NPAIRS, 2, L, E).transpose(0, 1, 3, 2).reshape(NPAIRS, 128, L)
        # v partition-major with the ones column baked in:
        # [pr, 128, 2, NB, 65];  v row 128*n+p of head (pr,h) -> [pr, p, h, n, 0:64]
        vx = np.ones((NPAIRS, 2, NB, 128, 65), np.float32).astype(BF16)
        vx[:, :, :, :, 0:64] = vc.reshape(NPAIRS, 2, NB, 128, E)
        vext = vx.transpose(0, 3, 1, 2, 4)  # [pr, 128, 2, NB, 65]
        # packed per-chunk input: for each chunk [q cols | k cols | v blocks]
        inpk = np.empty((NPAIRS, 128, PACK_COLS), BF16)
        for lo, hi in CHUNKS:
            n = hi - lo
            off = _chunk_off(lo)
            inpk[:, :, off : off + n * 128] = q2[:, :, lo * 128 : hi * 128]
            off += n * 128
            inpk[:, :, off : off + n * 128] = k2[:, :, lo * 128 : hi * 128]
            off += n * 128
            inpk[:, :, off : off + 2 * n * 65] = vext[:, :, :, lo:hi, :].reshape(
                NPAIRS, 128, 2 * n * 65
            )
        in_maps.append({"inpk": np.ascontiguousarray(inpk), "maskt": maskt})

    nc = _build_nc()
    res = run_bass_kernel_spmd(nc, in_maps, core_ids=list(range(NCORES)), trace=trace)
    # out_dev [pr, 128, NB, 2, 64] -> [pr, h, NB, 128, 64] -> [4, L, E]
    outs = np.stack(
        [
            np.asarray(res.results[c]["out"], dtype=np.float32)
            .transpose(0, 3, 2, 1, 4)
            .reshape(BH_PER_CORE, L, E)
            for c in range(NCORES)
        ]
    )
    full = outs.reshape(B, H, L, E)
    return full, res


def kernel(q, k, v, win_len):
    out, _ = _run(q, k, v, win_len, trace=False)
    return out


# revision 20
# speedup vs baseline: 1.3190x; 1.1939x over previous
"""Logsparse attention Trainium2 kernel.

Problem: B=4 H=8 L=4096 E=64, mask = causal & (dist <= win_len | dist is pow2).

Structure exploited: with 128-row query blocks b and 128-row key blocks,
query block b only interacts with key blocks {b, b-1, b-2, b-4, b-8, b-16}:
  - blocks b, b-1 carry the sliding window (win_len <= 127) plus pow2 dists
    {1..128} (dense-ish mask),
  - blocks b-2, b-4, b-8, b-16 carry exactly the pow2 dists 256/512/1024/2048,
    whose in-block mask is the pure diagonal kk == qq.

Sharding: B*H = 32 heads, 4 per core (8 cores). Heads processed in pairs:
the pair's [L, 2*64] q/k matrices are host-transposed into [128, L]
(e-on-partition) so QK^T matmuls contract over e; the two heads occupy
partition halves and run as row-packed (tile_position) concurrent matmuls.

Softmax: no max-subtraction (scores ~N(0,1), exp is safe in fp32/bf16);
denominator comes for free from a ones-column appended to V.
Compute dtype bf16 (inputs cast on host), accumulation fp32 (PSUM).

All HBM traffic is partition-major and contiguous per partition (v gets its
ones column and [128, 2, NB, 65] layout on the host; the output is stored
as [128, NB, 2, OUT_NB, 64] bf16 and un-permuted on the host) so DMAs don't
fragment into tiny descriptors. Input DMas are emitted need-first so the
first score matmul unblocks after ~3 small transfers.
"""

import os
import sys
from contextlib import ExitStack

import numpy as np

for _p in ("/opt/trn_rl_repo", "/root/.axon_site/_ro/trn_rl_repo"):
    if os.path.isdir(_p) and _p not in sys.path:
        sys.path.insert(0, _p)

import ml_dtypes  # noqa: E402
import concourse.bass as bass  # noqa: E402
import concourse.tile as tile  # noqa: E402
from concourse import bacc, mybir  # noqa: E402
from concourse.bass import ds  # noqa: E402
from concourse.bass_utils import run_bass_kernel_spmd  # noqa: E402

B, H, L, E = 4, 8, 4096, 64
NCORES = 8
BH = B * H                  # 32 heads total
BH_PER_CORE = BH // NCORES  # 4
NPAIRS = BH_PER_CORE // 2   # 2 head-pairs per core
NB = L // 128               # 32 query/key blocks
DELTAS = (0, 1, 2, 4, 8, 16)
NSLOT = len(DELTAS)
OUT_NB = 2                  # query blocks batched per output DMA
SCALE = 1.0 / float(np.sqrt(E))
BF16 = ml_dtypes.bfloat16

# input chunking (in 128-row blocks): small head chunks, need-ordered
CHUNKS = ((0, 2), (2, 4), (4, 8), (8, 20), (20, 32))
# columns per block in the packed per-chunk layout: q(128) + k(128) + v(2*65)
CPB = 128 + 128 + 2 * 65
PACK_COLS = NB * CPB

_NC_CACHE = {}


def _chunk_off(lo):
    return lo * CPB


def _chunk_lo(b):
    for lo, hi in CHUNKS:
        if lo <= b < hi:
            return lo
    raise ValueError(b)


def _active(b):
    return [(j, d) for j, d in enumerate(DELTAS) if b - d >= 0]


def _kernel_body(ctx, tc, inpk, maskt, out):
    nc = tc.nc
    consts = ctx.enter_context(tc.tile_pool(name="consts", bufs=1))
    pairbuf = ctx.enter_context(tc.tile_pool(name="pair", bufs=2))
    ppool = ctx.enter_context(tc.tile_pool(name="pexp", bufs=5))
    pmpool = ctx.enter_context(tc.tile_pool(name="pmask", bufs=5))
    spool = ctx.enter_context(tc.tile_pool(name="spsum", bufs=2, space="PSUM"))
    opool = ctx.enter_context(tc.tile_pool(name="opsum", bufs=2, space="PSUM"))
    rpool = ctx.enter_context(tc.tile_pool(name="rtile", bufs=4))
    outpool = ctx.enter_context(tc.tile_pool(name="outsb", bufs=3))

    # Input loads: q|k|v packed per chunk into ONE host-packed region so each
    # chunk is a single DMA trigger (HWDGE triggers cost ~0.6us each on the
    # sync queue and serialize). Per-chunk TILES keep dependency tracking
    # fine-grained: item 0's matmuls only wait for the first small chunk.
    # pr0's chunks are all emitted before pr1's (items are pr-major).
    qT_blk = [[None] * NB for _ in range(NPAIRS)]  # (tile, col_off) per block
    kT_blk = [[None] * NB for _ in range(NPAIRS)]
    v_blk = [[None] * NB for _ in range(NPAIRS)]  # (tile, col_off) per block
    mask_sb = consts.tile([128, 2, NSLOT, 128], mybir.dt.bfloat16)
    first = True
    for pr in range(NPAIRS):
        for lo, hi in CHUNKS:
            n = hi - lo
            sz = n * CPB
            t = pairbuf.tile([128, sz], mybir.dt.bfloat16, tag=f"in{pr}_{lo}")
            off = _chunk_off(lo)
            nc.sync.dma_start(out=t[:], in_=inpk[pr][:, ds(off, sz)])
            if first:
                # mask is first needed by item 0's expmask, after chunk 0
                nc.sync.dma_start(out=mask_sb[:], in_=maskt[:])
                first = False
            for b in range(lo, hi):
                i = b - lo
                qT_blk[pr][b] = (t, i * 128)
                kT_blk[pr][b] = (t, n * 128 + i * 128)
                # v block for head h at: 2*n*128 + h*(n*65) + i*65
                v_blk[pr][b] = (t, 2 * n * 128, n)

    # Software-pipelined emission (PE queue is in-order): scores of item t
    # are issued before exp/mask of t-1 and PV of t-2, so the PE always has
    # independent matmul work while ACT/DVE process earlier blocks.
    items = [(pr, b) for pr in range(NPAIRS) for b in range(NB)]
    st = {}
    out_sbs = {}

    def emit_scores(pr, b):
        S = spool.tile([128, 2, NSLOT, 128], mybir.dt.float32, tag="S")
        qt, qoff = qT_blk[pr][b]
        for j, d in _active(b):
            kt, koff = kT_blk[pr][b - d]
            for h in range(2):
                nc.tensor.matmul(
                    S[:, h, j, :],
                    lhsT=kt[64 * h : 64 * h + 64, ds(koff, 128)],
                    rhs=qt[64 * h : 64 * h + 64, ds(qoff, 128)],
                    start=True,
                    stop=True,
                    tile_position=(64 * h, 0),
                )
        st[(pr, b)] = S

    def emit_expmask(pr, b):
        S = st.pop((pr, b))
        # active slots form a contiguous prefix (DELTAS ascending; slot-major
        # flat layout): exp/mask only those, with plain 2D APs
        na = len(_active(b))
        P = ppool.tile([128, 2, NSLOT, 128], mybir.dt.bfloat16, tag="P")
        nc.scalar.activation(
            P[:, :, 0:na, :], S[:, :, 0:na, :],
            mybir.ActivationFunctionType.Exp, scale=SCALE,
        )
        PM = pmpool.tile([128, 2, NSLOT, 128], mybir.dt.bfloat16, tag="PM")
        nc.vector.tensor_mul(
            PM[:, :, 0:na, :], P[:, :, 0:na, :], mask_sb[:, :, 0:na, :]
        )
        st[(pr, b, "PM")] = PM

    def emit_pv(pr, b):
        PM = st.pop((pr, b, "PM"))
        acts = _active(b)
        O = opool.tile([128, 2, 65], mybir.dt.float32, tag="O")
        for h in range(2):
            for i, (j, d) in enumerate(acts):
                vt, vbase, n = v_blk[pr][b - d]
                voff = vbase + h * (n * 65) + (b - d - _chunk_lo(b - d)) * 65
                nc.tensor.matmul(
                    O[:, h, :],
                    lhsT=PM[:, h, j, :],
                    rhs=vt[:, ds(voff, 65)],
                    start=(i == 0),
                    stop=(i == len(acts) - 1),
                )
        r = rpool.tile([128, 2], mybir.dt.float32, tag="r")
        nc.vector.reciprocal(r[:], O[:, :, 64])
        if b % OUT_NB == 0:
            out_sbs[pr] = outpool.tile(
                [128, OUT_NB, 2, 64], mybir.dt.bfloat16, tag="osb", name="osb"
            )
        out_sb = out_sbs[pr]
        nc.vector.tensor_mul(
            out_sb[:, b % OUT_NB, :, :],
            O[:, :, 0:64],
            r[:].to_broadcast([128, 2, 64]),
        )
        if b % OUT_NB == OUT_NB - 1:
            w0 = b - (OUT_NB - 1)
            nc.sync.dma_start(
                out=out[pr][:, ds(w0, OUT_NB), :, :],
                in_=out_sb[:],
            )

    for t, (pr, b) in enumerate(items):
        emit_scores(pr, b)
        if t >= 1:
            emit_expmask(*items[t - 1])
        if t >= 2:
            emit_pv(*items[t - 2])
    emit_expmask(*items[-1])
    emit_pv(*items[-2])
    emit_pv(*items[-1])


def _build_nc():
    key = "v6r"
    if key in _NC_CACHE:
        return _NC_CACHE[key]
    nc = bacc.Bacc(
        "TRN2",
        target_bir_lowering=False,
        debug=False,
        enable_asserts=False,
        num_devices=NCORES,
    )
    inpk = nc.dram_tensor(
        "inpk", [NPAIRS, 128, PACK_COLS], mybir.dt.bfloat16, kind="ExternalInput"
    )
    maskt = nc.dram_tensor(
        "maskt", [128, 2, NSLOT, 128], mybir.dt.bfloat16, kind="ExternalInput"
    )
    out = nc.dram_tensor(
        "out", [NPAIRS, 128, NB, 2, 64], mybir.dt.bfloat16, kind="ExternalOutput"
    )
    with tile.TileContext(nc) as tc, ExitStack() as ctx:
        _kernel_body(ctx, tc, inpk.ap(), maskt.ap(), out.ap())
    nc.compile()
    _NC_CACHE[key] = nc
    return nc


def _mask_tiles(win):
    kk = np.arange(128, dtype=np.int64)[:, None]
    qq = np.arange(128, dtype=np.int64)[None, :]
    tiles = np.zeros((128, 2, NSLOT, 128), np.float32)
    for j, d in enumerate(DELTAS):
        dist = 128 * d + qq - kk
        pow2 = (dist > 0) & ((dist & (dist - 1)) == 0)
        ok = (dist >= 0) & ((dist <= win) | pow2)
        tiles[:, 0, j, :] = ok
        tiles[:, 1, j, :] = ok
    return tiles.astype(BF16)


def _run(q, k, v, win_len, trace=False):
    win = int(np.asarray(win_len))
    assert 0 <= win < 128, f"win_len {win} out of supported range [0, 128)"
    q = np.asarray(q, dtype=np.float32).reshape(BH, L, E)
    k = np.asarray(k, dtype=np.float32).reshape(BH, L, E)
    v = np.asarray(v, dtype=np.float32).reshape(BH, L, E)
    maskt = _mask_tiles(win)

    in_maps = []
    for c in range(NCORES):
        sl = slice(BH_PER_CORE * c, BH_PER_CORE * (c + 1))
        qc = q[sl].astype(BF16)  # [4, L, E]
        kc = k[sl].astype(BF16)
        vc = v[sl].astype(BF16)
        # pack head pairs on partitions, pre-transposed: [pairs, (h e), L]
        q2 = qc.reshape(NPAIRS, 2, L, E).transpose(0, 1, 3, 2).reshape(NPAIRS, 128, L)
        k2 = kc.reshape(NPAIRS, 2, L, E).transpose(0, 1, 3, 2).reshape(NPAIRS, 128, L)
        # v partition-major with the ones column baked in:
        # [pr, 128, 2, NB, 65];  v row 128*n+p of head (pr,h) -> [pr, p, h, n, 0:64]
        vx = np.ones((NPAIRS, 2, NB, 128, 65), np.float32).astype(BF16)
        vx[:, :, :, :, 0:64] = vc.reshape(NPAIRS, 2, NB, 128, E)
        vext = vx.transpose(0, 3, 1, 2, 4)  # [pr, 128, 2, NB, 65]
        # packed per-chunk input: for each chunk [q cols | k cols | v blocks]
        inpk = np.empty((NPAIRS, 128, PACK_COLS), BF16)
        for lo, hi in CHUNKS:
            n = hi - lo
            off = _chunk_off(lo)
            inpk[:, :, off : off + n * 128] = q2[:, :, lo * 128 : hi * 128]
            off += n * 128
            inpk[:, :, off : off + n * 128] = k2[:, :, lo * 128 : hi * 128]
            off += n * 128
            inpk[:, :, off : off + 2 * n * 65] = vext[:, :, :, lo:hi, :].reshape(
                NPAIRS, 128, 2 * n * 65
            )
        in_maps.append({"inpk": np.ascontiguousarray(inpk), "maskt": maskt})

    nc = _build_nc()
    res = run_bass_kernel_spmd(nc, in_maps, core_ids=list(range(NCORES)), trace=trace)
    # out_dev [pr, 128, NB, 2, 64] -> [pr, h, NB, 128, 64] -> [4, L, E]
    outs = np.stack(
        [
            np.asarray(res.results[c]["out"], dtype=np.float32)
            .transpose(0, 3, 2, 1, 4)
            .reshape(BH_PER_CORE, L, E)
            for c in range(NCORES)
        ]
    )
    full = outs.reshape(B, H, L, E)
    return full, res


def kernel(q, k, v, win_len):
    out, _ = _run(q, k, v, win_len, trace=False)
    return out
